# revision 3
# baseline (speedup 1.0000x reference)
"""Trainium2 Bass kernel v2 for nn_CDAN_Dis (CDAN discriminator head).

Math per sample m (see reference):
  a    = einsum('cf,bft->bct', w2d, feature)            # [C,T]
  d    = einsum('bct,bcpt->bpt', a, mask) + b2d         # [P,T]
  d    = leaky(GLN_scalar(d))                           # global LN over (P,T)
  x1   = leaky(GLN_vec(conv1d(d,  w1,b1, s2,p1)))       # [256,1000]
  x2   = leaky(GLN_vec(conv1d(x1, w2,b2, s2,p1)))       # [256,500]
  out  = conv1d(x2, w3, b3, s1, p0)                     # [1,500]

Sharding: data-parallel over batch M=4 across 4 NeuronCores.

v2 structure:
 - conv weights and conv activations (xpad/y1pad/x3) in bf16: halves the
   weight DMA and SBUF traffic; PSUM accumulation stays fp32.
 - inputs DMA'd in 4 T-chunks interleaved over the SP + Pool(SWDGE) rings
   (weights on the ACT ring first), so stage-1 compute pipelines with the
   input DMA instead of waiting for it.
 - GLN stat chains: per-partition partials -> one ones-matmul that both
   sums across partitions and broadcasts to all 128 -> [128,x] elementwise
   chain with Rsqrt (one table set 'reciprocal_sqrt_and_small' pinned).
 - conv-layer stats via bn_stats (one DVE pass per psum) + bn_aggr.
 - b2d is a uniform additive constant immediately followed by a global
   layernorm, so it cancels exactly and is ignored.
"""

import sys

sys.path.insert(0, "/opt/trn_rl_repo")

from contextlib import ExitStack

import numpy as np

import concourse.bass as bass
import concourse.mybir as mybir
import concourse.tile as tile
from concourse import bacc, bass_utils

F32 = mybir.dt.float32
F32R = mybir.dt.float32r
BF16 = mybir.dt.bfloat16
AX = mybir.AxisListType
OP = mybir.AluOpType
AF = mybir.ActivationFunctionType

M, C, B, T = 4, 2, 128, 2000
TC = 500               # chunk size (PSUM bank limit)
NCHUNK = T // TC       # 4
T1 = 1000              # conv1 output length
T2 = 500               # conv2 output length
EPS = 1e-8

N1 = B * T             # GLN1 element count

USE_PRELU = True       # fused affine+leaky on ACT (Prelu alpha=0.1)
WARM_MM = 2            # dummy matmuls per stage-1 chunk to trip the PE HAM
N_CORES = 4

# cwb: bf16 weights pack [128, CBW]
CB_W1T = 0             # 768 cols (k*256 + oh*128 + ocol)
CB_W2T = 768           # 1536 cols (cih*768 + k*256 + oh*128 + ocol)
CB_W3T = 2304          # 2 cols (+ 127 pad cols read as garbage lhsT)
CBW = 2434
# cwf: fp32 pack [128, CFW]
CF_W2DR = 0            # 256 cols: w2d[0] bcast | w2d[1] bcast
CF_ONES = 256          # 128 cols of 1.0 (stat sum+bcast matmul lhsT)
CF_G1 = 384            # [128,2]
CF_BB1 = 386
CF_G2 = 388
CF_BB2 = 390
CF_G2D = 392           # [128,1] replicated scalars
CF_BE2D = 393
CF_EPS = 394
CF_B1 = 395            # [128,2] conv1 bias (folded into GLN2 affine)
CF_B2 = 397            # [128,2] conv2 bias
CF_B3 = 399            # [128,1] replicated b3
CF_B1P = 400           # [128,8] conv1 bias in bn-stats group order
CF_B2P = 408           # [128,4] conv2 bias in bn-stats group order
CFW = 412


def R(ap):
    return ap.bitcast(F32R)


def F(ap):
    return ap.bitcast(F32)


def _patch_act_tables():
    """Pin every ACT func we use to the one set that has them all."""
    if getattr(bacc, "_cdan2_act_patch", False):
        return
    orig = bacc.get_activation_tables
    mine = {AF.Copy, AF.Identity, AF.Square, AF.Sqrt, AF.Prelu}

    def patched(arch):
        t = dict(orig(arch))
        for name in t:
            if name != "sqrt_and_others":
                t[name] = set(t[name]) - mine
        return t

    bacc.get_activation_tables = patched
    bacc._cdan2_act_patch = True


def build_nc(repeat=1):
    _patch_act_tables()
    nc = bacc.Bacc("TRN2", target_bir_lowering=False, debug=False,
                   num_devices=N_CORES)

    feature_d = nc.dram_tensor("feature", [B, T], F32, kind="ExternalInput").ap()
    mask_d = nc.dram_tensor("mask", [C, B, T], F32, kind="ExternalInput").ap()
    cwb_d = nc.dram_tensor("cwb", [128, CBW], BF16, kind="ExternalInput").ap()
    cwf_d = nc.dram_tensor("cwf", [128, CFW], F32, kind="ExternalInput").ap()
    out_d = nc.dram_tensor("out", [1, T2], F32, kind="ExternalOutput").ap()

    with tile.TileContext(nc) as tc:
        with ExitStack() as ctx:
            pools = _make_pools(ctx, tc)
            for _ in range(repeat):
                _build_kernel(pools, tc, feature_d, mask_d, cwb_d, cwf_d,
                              out_d)
    nc.compile()
    return nc


def _make_pools(ctx, tc):
    class P:
        pass
    p = P()
    p.const = ctx.enter_context(tc.tile_pool(name="const", bufs=2))
    p.inp = ctx.enter_context(tc.tile_pool(name="inp", bufs=2))
    p.tmpp = ctx.enter_context(tc.tile_pool(name="tmpp", bufs=2))
    p.sqp = ctx.enter_context(tc.tile_pool(name="sqp", bufs=2))
    p.bigp = ctx.enter_context(tc.tile_pool(name="bigp", bufs=2))
    p.smallp = ctx.enter_context(tc.tile_pool(name="smallp", bufs=2))
    p.psA = ctx.enter_context(tc.tile_pool(name="psA", bufs=2, space="PSUM"))
    p.psC = ctx.enter_context(tc.tile_pool(name="psC", bufs=4, space="PSUM"))
    p.psS = ctx.enter_context(tc.tile_pool(name="psS", bufs=2, space="PSUM"))
    return p


def _gln_tail(nc, smallp, me0, me1, eps_col, g_cols, bb_cols, tag,
              b_cols=None):
    """me0 = -mean (bcast [128,1]), me1 = -E[x^2] ([128,1]).
    Returns (alpha, beta) tiles [128, ncols] matching g_cols/bb_cols.
    If b_cols is given (conv bias folded into the affine), the normalize is
    applied to the bias-free psum y: beta = alpha*(b - mean) + bb."""
    sq = smallp.tile([128, 1], F32, tag=f"sq{tag}")
    nc.vector.tensor_mul(sq[:], me0, me0)                     # mean^2
    var = smallp.tile([128, 1], F32, tag=f"var{tag}")
    nc.vector.scalar_tensor_tensor(var[:], me1, -1.0, sq[:],
                                   OP.mult, OP.subtract)      # E2 - mean^2
    sstd = smallp.tile([128, 1], F32, tag=f"sstd{tag}")
    nc.scalar.activation(sstd[:], var[:], AF.Sqrt, bias=eps_col, scale=1.0)
    rstd = smallp.tile([128, 1], F32, tag=f"rstd{tag}")
    nc.vector.reciprocal(rstd[:], sstd[:])
    ncols = g_cols.shape[-1]
    alpha = smallp.tile([128, ncols], F32, tag=f"al{tag}")
    nc.vector.tensor_scalar_mul(alpha[:], g_cols, rstd[:])
    beta = smallp.tile([128, ncols], F32, tag=f"be{tag}")
    if b_cols is None:
        nc.vector.scalar_tensor_tensor(beta[:], alpha[:], me0, bb_cols,
                                       OP.mult, OP.add)       # -mean*a + bb
    else:
        bm = smallp.tile([128, ncols], F32, tag=f"bm{tag}")
        nc.vector.scalar_tensor_tensor(bm[:], b_cols, me0, alpha[:],
                                       OP.add, OP.mult)       # (b-mean)*a
        nc.vector.tensor_add(beta[:], bm[:], bb_cols)
    return alpha, beta


def _norm_leaky(nc, tmpp, out_ap, in_ap, scale_ap, bias_ap):
    """out = leaky(in*scale + bias), slope 0.1."""
    if USE_PRELU:
        nc.scalar.activation(out_ap, in_ap, AF.Prelu,
                             bias=bias_ap, scale=scale_ap, alpha=0.1)
    else:
        af = tmpp.tile([128, out_ap.shape[-1]], F32, tag="t")
        nc.scalar.activation(af[:], in_ap, AF.Identity,
                             bias=bias_ap, scale=scale_ap)
        nc.vector.scalar_tensor_tensor(out_ap, af[:], 0.1, af[:],
                                       OP.mult, OP.max)


def _build_kernel(pools, tc, feature_d, mask_d, cwb_d, cwf_d, out_d):
    nc = tc.nc
    const, inp, tmpp = pools.const, pools.inp, pools.tmpp
    sqp, bigp, smallp = pools.sqp, pools.bigp, pools.smallp
    psA, psC, psS = pools.psA, pools.psC, pools.psS

    # ---- DMA order tuned so chunk-0 inputs + w2dr land first.
    # SP ring: cwf, m0c0, f1, m0c1, f2, m0c2, m0c3
    # Pool (SWDGE): f0, m1c0, m1c1, m1c2, m1c3, f3
    # ACT ring: cwb (only needed from conv1 onward)
    w2drt = const.tile([128, 256], BF16, tag="w2dr")
    nc.sync.dma_start(w2drt[:], cwb_d[:, CB_W2DR:CB_W2DR + 256])
    cwf = const.tile([128, CFW], F32, tag="cwf")
    nc.sync.dma_start(cwf[:], cwf_d[:])
    cwb = const.tile([128, CB_W2DR], BF16, tag="cwb")
    nc.scalar.dma_start(cwb[:], cwb_d[:, 0:CB_W2DR])

    featc = [inp.tile([128, TC], F32, tag=f"feat{j}", name=f"feat{j}")
             for j in range(NCHUNK)]
    m0c = [inp.tile([128, TC], F32, tag=f"m0_{j}", name=f"m0_{j}")
           for j in range(NCHUNK)]
    m1c = [inp.tile([128, TC], F32, tag=f"m1_{j}", name=f"m1_{j}")
           for j in range(NCHUNK)]

    def csl(j):
        return slice(j * TC, (j + 1) * TC)

    nc.gpsimd.dma_start(featc[0][:], feature_d[:, csl(0)])
    nc.sync.dma_start(m0c[0][:], mask_d[0, :, csl(0)])
    nc.gpsimd.dma_start(m1c[0][:], mask_d[1, :, csl(0)])
    nc.sync.dma_start(featc[1][:], feature_d[:, csl(1)])
    nc.sync.dma_start(m0c[1][:], mask_d[0, :, csl(1)])
    nc.gpsimd.dma_start(m1c[1][:], mask_d[1, :, csl(1)])
    nc.sync.dma_start(featc[2][:], feature_d[:, csl(2)])
    nc.sync.dma_start(m0c[2][:], mask_d[0, :, csl(2)])
    nc.gpsimd.dma_start(m1c[2][:], mask_d[1, :, csl(2)])
    nc.gpsimd.dma_start(featc[3][:], feature_d[:, csl(3)])
    nc.sync.dma_start(m0c[3][:], mask_d[0, :, csl(3)])
    nc.gpsimd.dma_start(m1c[3][:], mask_d[1, :, csl(3)])

    # views into the packs
    w2dr = R(cwf[:, CF_W2DR:CF_W2DR + 256])
    onesT = R(cwf[:, CF_ONES:CF_ONES + 128])
    g1c = cwf[:, CF_G1:CF_G1 + 2]
    bb1c = cwf[:, CF_BB1:CF_BB1 + 2]
    g2c = cwf[:, CF_G2:CF_G2 + 2]
    bb2c = cwf[:, CF_BB2:CF_BB2 + 2]
    g2dc = cwf[:, CF_G2D:CF_G2D + 1]
    be2dc = cwf[:, CF_BE2D:CF_BE2D + 1]
    epsc = cwf[:, CF_EPS:CF_EPS + 1]
    b1c = cwf[:, CF_B1:CF_B1 + 2]
    b2c = cwf[:, CF_B2:CF_B2 + 2]
    b3c = cwf[0:1, CF_B3:CF_B3 + 1]
    b1p = cwf[:, CF_B1P:CF_B1P + 8]
    b2p = cwf[:, CF_B2P:CF_B2P + 4]

    d = bigp.tile([128, T], F32, tag="d")
    st1 = smallp.tile([128, 8], F32, tag="st1")   # [S1 x4 | S2 x4]

    # ---- stage 1: d = mask0*bcast(a0) + mask1*bcast(a1), fused stats ----
    for j in range(NCHUNK):
        sl = slice(j * TC, (j + 1) * TC)
        fR = R(featc[j][:])
        a0 = psA.tile([128, TC], F32, tag="a")
        nc.tensor.matmul(a0[:], w2dr[:, 0:128], fR, start=True, stop=True)
        a1 = psA.tile([128, TC], F32, tag="a")
        nc.tensor.matmul(a1[:], w2dr[:, 128:256], fR, start=True, stop=True)
        t0 = tmpp.tile([128, TC], F32, tag="t0")
        nc.vector.tensor_mul(t0[:], m0c[j][:], a0[:])
        t1 = tmpp.tile([128, TC], F32, tag="t1")
        nc.vector.tensor_mul(t1[:], m1c[j][:], a1[:])
        nc.vector.scalar_tensor_tensor(d[:, sl], t0[:], 0.0, t1[:],
                                       OP.add, OP.add,
                                       accum_out=st1[:, j:j + 1])
        sq = sqp.tile([128, TC], BF16, tag="sq")
        nc.scalar.activation(sq[:], d[:, sl], AF.Square,
                             accum_out=st1[:, 4 + j:5 + j])
        for _ in range(WARM_MM):
            wt = psC.tile([128, TC], F32, tag="c")
            nc.tensor.matmul(wt[:], w2dr[:, 0:128], fR, start=True, stop=True)

    # ---- GLN1: sum+bcast via ones-matmul, then [128,1] chain ----
    ps1 = psS.tile([128, 8], F32, tag="s")
    nc.tensor.matmul(ps1[:], onesT, R(st1[:]), start=True, stop=True)
    s12 = smallp.tile([128, 2], F32, tag="s12_1")
    nc.vector.reduce_sum(s12[:], ps1[:].rearrange("p (a b) -> p a b", a=2),
                         axis=AX.X)
    me1t = smallp.tile([128, 2], F32, tag="me1")
    nc.vector.tensor_scalar_mul(me1t[:], s12[:], -1.0 / N1)  # (-mean, -E2)
    al1, be1 = _gln_tail(nc, smallp, me1t[:, 0:1], me1t[:, 1:2],
                         epsc, g2dc, be2dc, "1")

    # ---- GLN1 normalize + leaky -> xpad (bf16) ----
    xpad = bigp.tile([128, T + 2], BF16, tag="xpad")
    nc.vector.memset(xpad[:, 0:1], 0.0)
    nc.vector.memset(xpad[:, T + 1:T + 2], 0.0)
    for j in range(2):
        sl = slice(j * 1000, (j + 1) * 1000)
        osl = slice(1 + j * 1000, 1 + (j + 1) * 1000)
        _norm_leaky(nc, tmpp, xpad[:, osl], d[:, sl], al1[:], be1[:])

    # ---- conv1 (128->256, k3 s2 p1); bias folded into GLN2 affine ----
    bn1 = smallp.tile([128, 24], F32, tag="bn1")
    py1 = {}
    for oh in range(2):
        for tcb in range(2):
            p = psC.tile([128, T2], F32, tag="c")
            py1[(oh, tcb)] = p
            for k in range(3):
                rhs = xpad[:, k + 2 * (tcb * T2):
                           k + 2 * (tcb * T2) + 2 * T2 - 1:2]
                nc.tensor.matmul(p[:], cwb[:, CB_W1T + k * 256 + oh * 128:
                                            CB_W1T + k * 256 + oh * 128 + 128],
                                 rhs, start=(k == 0), stop=(k == 2))
            idx = oh * 2 + tcb
            nc.vector.bn_stats(bn1[:, idx * 6:idx * 6 + 6], p[:])
    # shift the bn group means by the conv bias (z = y + b)
    nc.vector.tensor_add(bn1[:, 1:24:3], bn1[:, 1:24:3], b1p)

    # ---- GLN2: bn_aggr -> per-partition (mean,var) -> sum+bcast chain ----
    al2, be2 = _gln_bn_chain(nc, smallp, psS, onesT, epsc, bn1[:],
                             g1c, bb1c, "2", b_cols=b1c)

    # ---- GLN2 normalize + leaky -> y1pad (bf16) ----
    y1pad = []
    for oh in range(2):
        yp = bigp.tile([128, T1 + 2], BF16, tag=f"y1pad{oh}")
        y1pad.append(yp)
        nc.vector.memset(yp[:, 0:1], 0.0)
        nc.vector.memset(yp[:, T1 + 1:T1 + 2], 0.0)
        for tcb in range(2):
            osl = slice(1 + tcb * T2, 1 + (tcb + 1) * T2)
            _norm_leaky(nc, tmpp, yp[:, osl], py1[(oh, tcb)][:],
                        al2[:, oh:oh + 1], be2[:, oh:oh + 1])

    # ---- conv2 (256->256, k3 s2 p1); bias folded into GLN3 affine ----
    bn2 = smallp.tile([128, 12], F32, tag="bn2")
    py2 = {}
    for oh in range(2):
        p = psC.tile([128, T2], F32, tag="c")
        py2[oh] = p
        for cih in range(2):
            for k in range(3):
                rhs = y1pad[cih][:, k: k + 2 * T2 - 1:2]
                nc.tensor.matmul(p[:], cwb[:, CB_W2T + cih * 768 + k * 256 + oh * 128:
                                            CB_W2T + cih * 768 + k * 256 + oh * 128 + 128],
                                 rhs, start=(cih == 0 and k == 0),
                                 stop=(cih == 1 and k == 2))
        nc.vector.bn_stats(bn2[:, oh * 6:oh * 6 + 6], p[:])
    nc.vector.tensor_add(bn2[:, 1:12:3], bn2[:, 1:12:3], b2p)

    # ---- GLN3 ----
    al3, be3 = _gln_bn_chain(nc, smallp, psS, onesT, epsc, bn2[:],
                             g2c, bb2c, "3", b_cols=b2c)

    # ---- GLN3 normalize + leaky -> x3 halves (bf16) ----
    x3 = []
    for oh in range(2):
        xt = bigp.tile([128, T2], BF16, tag=f"x3_{oh}")
        x3.append(xt)
        _norm_leaky(nc, tmpp, xt[:], py2[oh][:],
                    al3[:, oh:oh + 1], be3[:, oh:oh + 1])

    # ---- conv3 (256->1, k1) + b3 ----
    # lhsT is 128 consecutive cwb columns whose col0 holds w3 for the half;
    # psum rows 1..127 accumulate garbage that we never read.
    p3 = psC.tile([128, T2], F32, tag="c")
    nc.tensor.matmul(p3[:], cwb[:, CB_W3T:CB_W3T + 128], x3[0][:],
                     start=True, stop=False)
    nc.tensor.matmul(p3[:], cwb[:, CB_W3T + 1:CB_W3T + 129], x3[1][:],
                     start=False, stop=True)
    out_s = smallp.tile([1, T2], F32, tag="out_s")
    nc.scalar.activation(out_s[:], p3[0:1, :], AF.Identity,
                         bias=b3c, scale=1.0)
    nc.sync.dma_start(out_d[:], out_s[:])


def _gln_bn_chain(nc, smallp, psS, onesT, eps_col, bn_ap, g_cols, bb_cols,
                  tag, b_cols=None):
    """bn_ap: [128, G*6] bn_stats groups. Produce (alpha, beta) [128,2]."""
    mvs = smallp.tile([128, 3], F32, tag=f"mvs{tag}")
    nc.vector.bn_aggr(mvs[:, 0:2], bn_ap)              # (mean_p, var_p)
    nc.vector.tensor_mul(mvs[:, 2:3], mvs[:, 0:1], mvs[:, 0:1])  # mean_p^2
    ps = psS.tile([128, 8], F32, tag="s")
    nc.tensor.matmul(ps[:, 0:3], onesT, R(mvs[:]), start=True, stop=True)
    me3 = smallp.tile([128, 3], F32, tag=f"me3{tag}")
    nc.vector.tensor_scalar_mul(me3[:], ps[:, 0:3], -1.0 / 128)
    me1 = smallp.tile([128, 1], F32, tag=f"me1{tag}")
    nc.vector.reduce_sum(me1[:], me3[:, 1:3].rearrange("p (a b) -> p a b", a=1),
                         axis=AX.X)                    # -(avg var + avg mean^2)
    return _gln_tail(nc, smallp, me3[:, 0:1], me1[:], eps_col,
                     g_cols, bb_cols, tag, b_cols=b_cols)


def shard_inputs(inputs):
    """Full inputs -> per-core in_maps (host-side layout prep)."""
    import ml_dtypes
    bf16 = ml_dtypes.bfloat16
    f = {k: np.ascontiguousarray(np.asarray(v, dtype=np.float32))
         for k, v in inputs.items()}

    cwb = np.zeros((128, CBW), np.float32)
    cwb[:, CB_W1T:CB_W1T + 768] = f["w1"].transpose(1, 2, 0).reshape(128, 768)
    cwb[:, CB_W2T:CB_W2T + 1536] = (
        f["w2"].transpose(1, 2, 0).reshape(2, 128, 3, 256)
        .transpose(1, 0, 2, 3).reshape(128, 1536))
    cwb[:, CB_W3T:CB_W3T + 2] = f["w3"].reshape(2, 128).T
    cwb = cwb.astype(bf16)

    cwf = np.zeros((128, CFW), np.float32)
    w2d = f["w2d"]
    cwf[:, CF_W2DR:CF_W2DR + 128] = np.tile(w2d[0][:, None], (1, 128))
    cwf[:, CF_W2DR + 128:CF_W2DR + 256] = np.tile(w2d[1][:, None], (1, 128))
    cwf[:, CF_ONES:CF_ONES + 128] = 1.0
    cwf[:, CF_G1:CF_G1 + 2] = f["g1"].reshape(2, 128).T
    cwf[:, CF_BB1:CF_BB1 + 2] = f["bb1"].reshape(2, 128).T
    cwf[:, CF_G2:CF_G2 + 2] = f["g2"].reshape(2, 128).T
    cwf[:, CF_BB2:CF_BB2 + 2] = f["bb2"].reshape(2, 128).T
    cwf[:, CF_G2D] = float(f["g2d"].reshape(()))
    cwf[:, CF_BE2D] = float(f["be2d"].reshape(()))
    cwf[:, CF_EPS] = EPS
    b1t = f["b1"].reshape(2, 128).T          # [128, 2] by half
    b2t = f["b2"].reshape(2, 128).T
    cwf[:, CF_B1:CF_B1 + 2] = b1t
    cwf[:, CF_B2:CF_B2 + 2] = b2t
    cwf[:, CF_B3] = float(f["b3"].reshape(()))
    # bn group-mean corrections, group order idx = oh*2 + tcb, (even|odd)
    cwf[:, CF_B1P:CF_B1P + 8] = np.repeat(b1t, 4, axis=1)
    cwf[:, CF_B2P:CF_B2P + 4] = np.repeat(b2t, 2, axis=1)

    in_maps = []
    for i in range(M):
        in_maps.append(dict(cwb=cwb, cwf=cwf,
                            feature=np.ascontiguousarray(f["feature"][i]),
                            mask=np.ascontiguousarray(f["mask"][i])))
    return in_maps


_NC = None


def kernel(**inputs):
    global _NC
    if _NC is None:
        _NC = build_nc()
    in_maps = shard_inputs(inputs)
    res = bass_utils.run_bass_kernel_spmd(_NC, in_maps,
                                          core_ids=list(range(N_CORES)))
    out = np.stack([res.results[i]["out"] for i in range(M)], axis=0)
    return out.astype(np.float32)
